# revision 2
# baseline (speedup 1.0000x reference)
"""PillarMaxPoolingV2a on 8 TRN2 NeuronCores (Bass/Tile) — v2.

Strategy (batch-parallel across 8 cores, scatter-free segment max):
  Host:
    - mirror the reference's pillar-index math exactly (f32)
    - fold BatchNorm scale into the 1x1-conv weights (bias applied on host)
    - per core (= one batch): group points by pillar into groups of 8
      (padding replicates a real member), lay points out in a 4-lane
      channel-major bf16 stream
    - device returns per-group channel-major maxima (pre-bias, bf16);
      host combines overflow groups, applies bias+ReLU, masks empties
  Device (per core):
    - feats resident in SBUF as one [128, NB*512] bf16 tile
      (partitions = 4 lanes x 32 input channels)
    - block-diagonal weights: one matmul computes 2 lanes' outputs
      (contraction 64, out 128 = 2 lanes x 64 channels, F=512)
    - PSUM drain split across engines (pattern-tunable):
        D  = DVE tensor_reduce (max over 8 members) direct from PSUM
        A  = ACT copy PSUM->SBUF bf16, then DVE 3-level max tree (2x mode)
        P  = ACT copy PSUM->SBUF bf16, then Pool(gpsimd) 3-level max tree
    - per-unit [128, n*64] bf16 stage -> DMA out
"""
import math
import numpy as np
import sys

sys.path.insert(0, "/opt/trn_rl_repo")

import ml_dtypes

BF16 = ml_dtypes.bfloat16

# ---- problem constants (hardcoded per contract) ----
B = 8
NPOINTS = 1_600_000
C_IN = 29
C_OUT = 64
BEV = np.float32(0.8)
X_MIN = np.float32(0.0)
Y_MIN = np.float32(-40.0)
W = 88
H = 100
EPS = np.float32(1e-5)
NPIL = H * W            # 8800 pillars per batch
GK = 8                  # points per group

# drain-engine pattern over units (tunable; D=DVE direct, A=ACT+DVE tree)
PATTERN = "AAAAAAAAD"

_prog_cache = {}
_debug_state = {}


def _build_program(NB: int, pattern: str):
    """Device program: NB col-slices (512 cols x 4 lanes = 2048 points each)."""
    import concourse.bass as bass
    import concourse.bacc as bacc
    import concourse.mybir as mybir
    import concourse.tile as tile

    NCH4 = (NB + 3) // 4     # output super-chunks of up to 4 col-slices
    NDMA_CS = 4              # col-slices per input DMA

    nc = bacc.Bacc("TRN2", target_bir_lowering=False, debug=False, num_devices=8)
    feats_in = nc.declare_dram_parameter("feats", [128, NB * 512], mybir.dt.bfloat16, isOutput=False)
    w_in = nc.declare_dram_parameter("wblk", [128, 128], mybir.dt.bfloat16, isOutput=False)
    out = nc.declare_dram_parameter("outr", [128, NB * 128], mybir.dt.bfloat16, isOutput=True)

    with tile.TileContext(nc) as tc:
        with (
            tc.tile_pool(name="const", bufs=1) as constp,
            tc.tile_pool(name="fbig", bufs=1) as fbigp,
            tc.tile_pool(name="evac", bufs=6) as evacp,
            tc.tile_pool(name="tree1", bufs=6) as tree1p,
            tc.tile_pool(name="tree2", bufs=6) as tree2p,
            tc.tile_pool(name="stage", bufs=6) as stagep,
            tc.tile_pool(name="psum", bufs=4, space="PSUM") as psump,
        ):
            wt = constp.tile([128, 128], mybir.dt.bfloat16)
            nc.sync.dma_start(out=wt[:], in_=w_in[:])
            fbig = fbigp.tile([128, NB * 512], mybir.dt.bfloat16)
            # first DMAs kept small so the first matmuls start early; all
            # input DMAs dispatch from the otherwise-idle GpSimd SWDGE queue,
            # keeping SP free for the per-chunk output DMAs
            bounds = [0, 1, 3] + list(range(3 + NDMA_CS, NB, NDMA_CS)) + [NB]
            for d0, d1 in zip(bounds[:-1], bounds[1:]):
                if d1 > d0:
                    nc.gpsimd.dma_start(out=fbig[:, d0 * 512:d1 * 512],
                                        in_=feats_in[:, d0 * 512:d1 * 512])

            # PE warm-up: ~7us of matmuls on the first feats slice while the
            # bulk input DMAs land, so HAM un-throttles before the real work
            wps = psump.tile([128, 1024], mybir.dt.float32, tag="ps")
            for w in range(24):
                nc.tensor.matmul(
                    out=wps[:, (w % 2) * 512:(w % 2) * 512 + 512],
                    lhsT=wt[0:64, :], rhs=fbig[0:64, 0:512],
                    start=True, stop=True,
                )

            u = 0
            for k4 in range(NCH4):
                n4 = min(4, NB - 4 * k4)
                stg0 = stagep.tile([128, 256], mybir.dt.bfloat16, tag="stg0")
                stg1 = stagep.tile([128, 256], mybir.dt.bfloat16, tag="stg1")
                stgs_pr = [stg0, stg1]
                for h in range(2):              # 2-col-slice units
                    cs0 = 4 * k4 + 2 * h
                    n = min(2, NB - cs0)
                    if n <= 0:
                        continue
                    for pr in range(2):         # alternate PE row groups
                        ps = psump.tile([128, 1024], mybir.dt.float32, tag="ps")
                        for i in range(n):
                            nc.tensor.matmul(
                                out=ps[:, i * 512:(i + 1) * 512],
                                lhsT=wt[pr * 64:pr * 64 + 64, :],
                                rhs=fbig[pr * 64:pr * 64 + 64,
                                         (cs0 + i) * 512:(cs0 + i + 1) * 512],
                                start=True, stop=True,
                            )
                        stg = stgs_pr[pr]
                        stgs = stg[:, 2 * h * 64:(2 * h + n) * 64]
                        eng = pattern[u % len(pattern)]
                        u += 1
                        if eng == "D":
                            nc.vector.tensor_reduce(
                                out=stgs.rearrange("p (i g) -> p i g", g=64),
                                in_=ps[:, 0:n * 512].rearrange(
                                    "p (i m g) -> p i g m", m=8, g=64),
                                axis=mybir.AxisListType.X, op=mybir.AluOpType.max)
                        else:
                            # ACT evacuates to bf16 SBUF, permuting member-
                            # halves into the two big halves of the tile:
                            # col = a*(n*256) + i*256 + v  (a = member div 4)
                            ev = evacp.tile([128, 1024], mybir.dt.bfloat16, tag="ev")
                            nc.scalar.copy(
                                out=ev[:, 0:n * 512].rearrange(
                                    "p (a i v) -> p i a v", a=2, v=256),
                                in_=ps[:, 0:n * 512].rearrange(
                                    "p (i a v) -> p i a v", a=2, v=256))
                            te = nc.vector
                            t1 = tree1p.tile([128, 512], mybir.dt.bfloat16, tag="t1")
                            te.tensor_max(
                                out=t1[:, 0:n * 256].rearrange(
                                    "p (b i v) -> p i b v", b=2, v=128),
                                in0=ev[:, 0:n * 256].rearrange(
                                    "p (i b v) -> p i b v", b=2, v=128),
                                in1=ev[:, n * 256:2 * n * 256].rearrange(
                                    "p (i b v) -> p i b v", b=2, v=128))
                            t2 = tree2p.tile([128, 256], mybir.dt.bfloat16, tag="t2")
                            te.tensor_max(
                                out=t2[:, 0:n * 128].rearrange(
                                    "p (c i g) -> p i c g", c=2, g=64),
                                in0=t1[:, 0:n * 128].rearrange(
                                    "p (i c g) -> p i c g", c=2, g=64),
                                in1=t1[:, n * 128:2 * n * 128].rearrange(
                                    "p (i c g) -> p i c g", c=2, g=64))
                            te.tensor_max(
                                out=stgs.rearrange("p (i g) -> p i g", g=64),
                                in0=t2[:, 0:n * 64].rearrange("p (i g) -> p i g", g=64),
                                in1=t2[:, n * 64:2 * n * 64].rearrange(
                                    "p (i g) -> p i g", g=64))
                for pr in range(2):
                    base = 512 * k4 + pr * 64 * n4
                    nc.sync.dma_start(out=out[:, base:base + n4 * 64],
                                      in_=stgs_pr[pr][:, 0:n4 * 64])
    nc.compile()
    return nc


def _group_layout(pid, counts):
    """Per-core group construction. Returns (src, n_ov, ov_pillar, n_groups):
    src[j, m] = point index feeding member m of group j, groups ordered with
    the first NPIL in pillar order, overflow groups appended (pillar-sorted)."""
    n = pid.shape[0]
    order = np.argsort(pid, kind="stable")
    starts = np.zeros(NPIL, dtype=np.int64)
    np.cumsum(counts[:-1], out=starts[1:])

    gcnt = np.maximum((counts + GK - 1) // GK, 1)
    n_ov_per = gcnt - 1
    n_ov = int(n_ov_per.sum())
    n_groups = NPIL + n_ov

    grp_pillar = np.empty(n_groups, dtype=np.int64)
    grp_rank = np.empty(n_groups, dtype=np.int64)
    grp_pillar[:NPIL] = np.arange(NPIL)
    grp_rank[:NPIL] = 0
    ov_pillar = np.repeat(np.arange(NPIL), n_ov_per)
    grp_pillar[NPIL:] = ov_pillar
    ov_rank = np.arange(n_ov) - np.repeat(np.cumsum(n_ov_per) - n_ov_per, n_ov_per) + 1
    grp_rank[NPIL:] = ov_rank

    cnt_g = counts[grp_pillar]
    base = starts[grp_pillar]
    m = np.arange(GK)
    offs = np.minimum(grp_rank[:, None] * GK + m[None, :],
                      np.maximum(cnt_g - 1, 0)[:, None])
    srcpos = np.where((cnt_g > 0)[:, None], base[:, None] + offs, 0)
    src = order[srcpos] if n > 0 else np.zeros((n_groups, GK), np.int64)
    return src, n_ov, ov_pillar, n_groups


def _slot_maps(NB: int):
    """Static slot -> (lane, featcol base, outcol) maps for NB col-slices."""
    s = np.arange(NB * 256)
    cs = s // 256
    r = s % 256
    lane = r // 64
    g = r % 64
    k = cs // 4
    n = np.minimum(4, NB - 4 * k)
    outcol = 512 * k + (lane // 2) * 64 * n + (cs - 4 * k) * 64 + g
    featbase = cs * 512 + g          # member m at featbase + 64*m
    return lane.astype(np.int32), featbase.astype(np.int64), outcol.astype(np.int64)


def kernel(xyz, xyz_batch_cnt, point_features, conv_w, bn_gamma, bn_beta,
           bn_mean, bn_var, _emulate=False):
    xyz = np.asarray(xyz, dtype=np.float32)
    cnt = np.asarray(xyz_batch_cnt, dtype=np.int64)
    pf = np.asarray(point_features, dtype=np.float32)
    conv_w = np.asarray(conv_w, dtype=np.float32)
    bn_gamma = np.asarray(bn_gamma, dtype=np.float32)
    bn_beta = np.asarray(bn_beta, dtype=np.float32)
    bn_mean = np.asarray(bn_mean, dtype=np.float32)
    bn_var = np.asarray(bn_var, dtype=np.float32)
    N = xyz.shape[0]

    # --- batch ids exactly like jnp.repeat(..., total_repeat_length=N) ---
    ids = np.repeat(np.arange(B), np.maximum(cnt, 0))
    if ids.shape[0] < N:
        pad_val = ids[-1] if ids.shape[0] else 0
        ids = np.concatenate([ids, np.full(N - ids.shape[0], pad_val, np.int64)])
    ids = ids[:N]

    # --- pillar index math, mirroring the reference in f32 ---
    ix = np.clip(np.floor((xyz[:, 0] - X_MIN) / BEV).astype(np.int32), 0, W - 1)
    iy = np.clip(np.floor((xyz[:, 1] - Y_MIN) / BEV).astype(np.int32), 0, H - 1)
    pid_local = iy.astype(np.int64) * W + ix.astype(np.int64)
    cx = (ix.astype(np.float32) + np.float32(0.5)) * BEV + X_MIN
    cy = (iy.astype(np.float32) + np.float32(0.5)) * BEV + Y_MIN
    feats = np.empty((N, 32), dtype=np.float32)
    feats[:, 0] = xyz[:, 0] - cx
    feats[:, 1] = xyz[:, 1] - cy
    feats[:, 2] = xyz[:, 2]
    feats[:, 3:] = pf

    # --- BN fold (scale only; bias applied on host after max) ---
    s = bn_gamma / np.sqrt(bn_var + EPS)
    wt32 = (conv_w * s[:, None]).T                          # [32, 64]
    b2 = bn_beta - bn_mean * s                              # [64]
    wblk = np.zeros((128, 128), dtype=np.float32)
    wblk[0:32, 0:64] = wt32
    wblk[32:64, 64:128] = wt32
    wblk[64:96, 0:64] = wt32
    wblk[96:128, 64:128] = wt32
    wblk_bf = wblk.astype(BF16)

    # --- per-core grouping ---
    bounds = np.searchsorted(ids, np.arange(B + 1))
    cores = []
    max_groups = 0
    for c in range(B):
        lo, hi = int(bounds[c]), int(bounds[c + 1])
        pidc = pid_local[lo:hi]
        counts = np.bincount(pidc, minlength=NPIL).astype(np.int64)
        src, n_ov, ov_pillar, n_groups = _group_layout(pidc, counts)
        cores.append((lo, hi, src, n_ov, ov_pillar, n_groups, counts))
        max_groups = max(max_groups, n_groups)

    NB = max(1, math.ceil(max_groups / 256))
    lane_map, featbase_map, outcol_map = _slot_maps(NB)
    ncols = NB * 512

    # --- pack per-core inputs ---
    in_maps = []
    emu_feats = []
    for c in range(B):
        lo, hi, src, n_ov, ov_pillar, n_groups, counts = cores[c]
        fc = feats[lo:hi]
        if fc.shape[0] == 0:
            fc = np.zeros((1, 32), dtype=np.float32)
        lanes = lane_map[:n_groups]
        fcol = (featbase_map[:n_groups, None] + 64 * np.arange(GK)[None, :])
        fdev = np.zeros((128, ncols), dtype=BF16)
        for L in range(4):
            P = np.zeros(ncols, dtype=np.int64)
            mask = lanes == L
            P[fcol[mask].ravel()] = src[:n_groups][mask].ravel()
            fdev[L * 32:(L + 1) * 32, :] = \
                np.ascontiguousarray(fc[P].T).astype(BF16)
        in_maps.append({"feats": fdev, "wblk": wblk_bf})
        if _emulate:
            emu_feats.append(fdev)

    # --- run (device or numpy emulation) ---
    if _emulate:
        results = []
        for c in range(B):
            fd = emu_feats[c].astype(np.float32)           # [128, ncols]
            o = np.zeros((128, NB * 128), dtype=np.float32)
            for pr in range(2):
                h = wblk[pr * 64:pr * 64 + 64, :].T @ fd[pr * 64:pr * 64 + 64, :]
                # h: [128, ncols]; reduce over members (stride-64 blocks)
                hv = h.reshape(128, NB, 8, 64).max(axis=2)  # [128, NB, 64]
                hv = hv.astype(BF16).astype(np.float32)
                for cs in range(NB):
                    k = cs // 4
                    n = min(4, NB - 4 * k)
                    base = 512 * k + pr * 64 * n + (cs - 4 * k) * 64
                    o[:, base:base + 64] = hv[:, cs, :]
            results.append({"outr": o.astype(BF16)})
    else:
        from concourse.bass_utils import run_bass_kernel_spmd
        key = (NB, PATTERN)
        if key not in _prog_cache:
            _prog_cache[key] = _build_program(NB, PATTERN)
        nc = _prog_cache[key]
        _debug_state["nc"] = nc
        _debug_state["in_maps"] = in_maps
        res = run_bass_kernel_spmd(nc, in_maps, core_ids=list(range(B)))
        results = res.results

    # --- unpack + combine ---
    out_full = np.zeros((B * NPIL, C_OUT), dtype=np.float32)
    for c in range(B):
        lo, hi, src, n_ov, ov_pillar, n_groups, counts = cores[c]
        o = np.asarray(results[c]["outr"]).astype(np.float32)  # [128, NB*128]
        lanes = lane_map[:n_groups]
        ocols = outcol_map[:n_groups]
        rows = np.empty((n_groups, 64), dtype=np.float32)
        for lb in range(2):
            mask = (lanes % 2) == lb
            rows[mask] = o[lb * 64:(lb + 1) * 64, ocols[mask]].T
        main = rows[:NPIL].copy()
        if n_ov:
            ov = rows[NPIL:NPIL + n_ov]
            runs = np.flatnonzero(np.diff(ov_pillar, prepend=-1))
            red = np.maximum.reduceat(ov, runs, axis=0)
            upid = ov_pillar[runs]
            main[upid] = np.maximum(main[upid], red)
        outc = np.maximum(main + b2[None, :], np.float32(0.0))
        outc[counts == 0] = 0.0
        out_full[c * NPIL:(c + 1) * NPIL] = outc
    return out_full


# revision 3
# speedup vs baseline: 1.1165x; 1.1165x over previous
"""PillarMaxPoolingV2a on 8 TRN2 NeuronCores (Bass/Tile) — v2.

Strategy (batch-parallel across 8 cores, scatter-free segment max):
  Host:
    - mirror the reference's pillar-index math exactly (f32)
    - fold BatchNorm scale into the 1x1-conv weights (bias applied on host)
    - per core (= one batch): group points by pillar into groups of 8
      (padding replicates a real member), lay points out in a 4-lane
      channel-major bf16 stream
    - device returns per-group channel-major maxima (pre-bias, bf16);
      host combines overflow groups, applies bias+ReLU, masks empties
  Device (per core):
    - feats resident in SBUF as one [128, NB*512] bf16 tile
      (partitions = 4 lanes x 32 input channels)
    - block-diagonal weights: one matmul computes 2 lanes' outputs
      (contraction 64, out 128 = 2 lanes x 64 channels, F=512)
    - PSUM drain split across engines (pattern-tunable):
        D  = DVE tensor_reduce (max over 8 members) direct from PSUM
        A  = ACT copy PSUM->SBUF bf16, then DVE 3-level max tree (2x mode)
        P  = ACT copy PSUM->SBUF bf16, then Pool(gpsimd) 3-level max tree
    - per-unit [128, n*64] bf16 stage -> DMA out
"""
import math
import numpy as np
import sys

sys.path.insert(0, "/opt/trn_rl_repo")

import ml_dtypes

BF16 = ml_dtypes.bfloat16

# ---- problem constants (hardcoded per contract) ----
B = 8
NPOINTS = 1_600_000
C_IN = 29
C_OUT = 64
BEV = np.float32(0.8)
X_MIN = np.float32(0.0)
Y_MIN = np.float32(-40.0)
W = 88
H = 100
EPS = np.float32(1e-5)
NPIL = H * W            # 8800 pillars per batch
GK = 8                  # points per group

# drain-engine pattern over units (tunable; D=DVE direct, A=ACT+DVE tree)
PATTERN = "AAAAAAAAD"

_prog_cache = {}
_debug_state = {}


def _build_program(NB: int, pattern: str):
    """Device program: NB col-slices (512 cols x 4 lanes = 2048 points each)."""
    import concourse.bass as bass
    import concourse.bacc as bacc
    import concourse.mybir as mybir
    import concourse.tile as tile

    NCH4 = (NB + 3) // 4     # output super-chunks of up to 4 col-slices
    NDMA_CS = 4              # col-slices per input DMA

    nc = bacc.Bacc("TRN2", target_bir_lowering=False, debug=False, num_devices=8)
    feats_in = nc.declare_dram_parameter("feats", [128, NB * 512], mybir.dt.bfloat16, isOutput=False)
    w_in = nc.declare_dram_parameter("wblk", [128, 128], mybir.dt.bfloat16, isOutput=False)
    out = nc.declare_dram_parameter("outr", [128, NB * 128], mybir.dt.bfloat16, isOutput=True)

    with tile.TileContext(nc) as tc:
        with (
            tc.tile_pool(name="const", bufs=1) as constp,
            tc.tile_pool(name="fbig", bufs=1) as fbigp,
            tc.tile_pool(name="evac", bufs=6) as evacp,
            tc.tile_pool(name="tree1", bufs=6) as tree1p,
            tc.tile_pool(name="tree2", bufs=6) as tree2p,
            tc.tile_pool(name="stage", bufs=6) as stagep,
            tc.tile_pool(name="psum", bufs=4, space="PSUM") as psump,
        ):
            wt = constp.tile([128, 128], mybir.dt.bfloat16)
            nc.sync.dma_start(out=wt[:], in_=w_in[:])
            fbig = fbigp.tile([128, NB * 512], mybir.dt.bfloat16)
            # first DMAs kept small so the first matmuls start early; all
            # input DMAs dispatch from the otherwise-idle GpSimd SWDGE queue,
            # keeping SP free for the per-chunk output DMAs
            bounds = [0, 1, 3] + list(range(3 + NDMA_CS, NB, NDMA_CS)) + [NB]
            for d0, d1 in zip(bounds[:-1], bounds[1:]):
                if d1 > d0:
                    nc.gpsimd.dma_start(out=fbig[:, d0 * 512:d1 * 512],
                                        in_=feats_in[:, d0 * 512:d1 * 512])

            # PE warm-up: ~4us of small matmuls on the weight tile itself
            # (no feats dependency), so HAM un-throttles while input DMAs land
            wps = psump.tile([128, 1024], mybir.dt.float32, tag="ps")
            for w in range(36):
                nc.tensor.matmul(
                    out=wps[:, (w % 8) * 128:(w % 8) * 128 + 128],
                    lhsT=wt[0:64, :], rhs=wt[0:64, :],
                    start=True, stop=True,
                )

            u = 0
            for k4 in range(NCH4):
                n4 = min(4, NB - 4 * k4)
                stg0 = stagep.tile([128, 256], mybir.dt.bfloat16, tag="stg0")
                stg1 = stagep.tile([128, 256], mybir.dt.bfloat16, tag="stg1")
                stgs_pr = [stg0, stg1]
                for h in range(2):              # 2-col-slice units
                    cs0 = 4 * k4 + 2 * h
                    n = min(2, NB - cs0)
                    if n <= 0:
                        continue
                    for pr in range(2):         # alternate PE row groups
                        ps = psump.tile([128, 1024], mybir.dt.float32, tag="ps")
                        for i in range(n):
                            nc.tensor.matmul(
                                out=ps[:, i * 512:(i + 1) * 512],
                                lhsT=wt[pr * 64:pr * 64 + 64, :],
                                rhs=fbig[pr * 64:pr * 64 + 64,
                                         (cs0 + i) * 512:(cs0 + i + 1) * 512],
                                start=True, stop=True,
                            )
                        stg = stgs_pr[pr]
                        stgs = stg[:, 2 * h * 64:(2 * h + n) * 64]
                        eng = pattern[u % len(pattern)]
                        u += 1
                        if eng == "D":
                            nc.vector.tensor_reduce(
                                out=stgs.rearrange("p (i g) -> p i g", g=64),
                                in_=ps[:, 0:n * 512].rearrange(
                                    "p (i m g) -> p i g m", m=8, g=64),
                                axis=mybir.AxisListType.X, op=mybir.AluOpType.max)
                        else:
                            # ACT evacuates to bf16 SBUF, permuting member-
                            # halves into the two big halves of the tile:
                            # col = a*(n*256) + i*256 + v  (a = member div 4)
                            ev = evacp.tile([128, 1024], mybir.dt.bfloat16, tag="ev")
                            nc.scalar.copy(
                                out=ev[:, 0:n * 512].rearrange(
                                    "p (a i v) -> p i a v", a=2, v=256),
                                in_=ps[:, 0:n * 512].rearrange(
                                    "p (i a v) -> p i a v", a=2, v=256))
                            te = nc.vector
                            t1 = tree1p.tile([128, 512], mybir.dt.bfloat16, tag="t1")
                            te.tensor_max(
                                out=t1[:, 0:n * 256].rearrange(
                                    "p (b i v) -> p i b v", b=2, v=128),
                                in0=ev[:, 0:n * 256].rearrange(
                                    "p (i b v) -> p i b v", b=2, v=128),
                                in1=ev[:, n * 256:2 * n * 256].rearrange(
                                    "p (i b v) -> p i b v", b=2, v=128))
                            t2 = tree2p.tile([128, 256], mybir.dt.bfloat16, tag="t2")
                            te.tensor_max(
                                out=t2[:, 0:n * 128].rearrange(
                                    "p (c i g) -> p i c g", c=2, g=64),
                                in0=t1[:, 0:n * 128].rearrange(
                                    "p (i c g) -> p i c g", c=2, g=64),
                                in1=t1[:, n * 128:2 * n * 128].rearrange(
                                    "p (i c g) -> p i c g", c=2, g=64))
                            te.tensor_max(
                                out=stgs.rearrange("p (i g) -> p i g", g=64),
                                in0=t2[:, 0:n * 64].rearrange("p (i g) -> p i g", g=64),
                                in1=t2[:, n * 64:2 * n * 64].rearrange(
                                    "p (i g) -> p i g", g=64))
                for pr in range(2):
                    base = 512 * k4 + pr * 64 * n4
                    nc.sync.dma_start(out=out[:, base:base + n4 * 64],
                                      in_=stgs_pr[pr][:, 0:n4 * 64])
    nc.compile()
    return nc


def _group_layout(pid, counts):
    """Per-core group construction. Returns (src, n_ov, ov_pillar, n_groups):
    src[j, m] = point index feeding member m of group j, groups ordered with
    the first NPIL in pillar order, overflow groups appended (pillar-sorted)."""
    n = pid.shape[0]
    order = np.argsort(pid, kind="stable")
    starts = np.zeros(NPIL, dtype=np.int64)
    np.cumsum(counts[:-1], out=starts[1:])

    gcnt = np.maximum((counts + GK - 1) // GK, 1)
    n_ov_per = gcnt - 1
    n_ov = int(n_ov_per.sum())
    n_groups = NPIL + n_ov

    grp_pillar = np.empty(n_groups, dtype=np.int64)
    grp_rank = np.empty(n_groups, dtype=np.int64)
    grp_pillar[:NPIL] = np.arange(NPIL)
    grp_rank[:NPIL] = 0
    ov_pillar = np.repeat(np.arange(NPIL), n_ov_per)
    grp_pillar[NPIL:] = ov_pillar
    ov_rank = np.arange(n_ov) - np.repeat(np.cumsum(n_ov_per) - n_ov_per, n_ov_per) + 1
    grp_rank[NPIL:] = ov_rank

    cnt_g = counts[grp_pillar]
    base = starts[grp_pillar]
    m = np.arange(GK)
    offs = np.minimum(grp_rank[:, None] * GK + m[None, :],
                      np.maximum(cnt_g - 1, 0)[:, None])
    srcpos = np.where((cnt_g > 0)[:, None], base[:, None] + offs, 0)
    src = order[srcpos] if n > 0 else np.zeros((n_groups, GK), np.int64)
    return src, n_ov, ov_pillar, n_groups


def _slot_maps(NB: int):
    """Static slot -> (lane, featcol base, outcol) maps for NB col-slices."""
    s = np.arange(NB * 256)
    cs = s // 256
    r = s % 256
    lane = r // 64
    g = r % 64
    k = cs // 4
    n = np.minimum(4, NB - 4 * k)
    outcol = 512 * k + (lane // 2) * 64 * n + (cs - 4 * k) * 64 + g
    featbase = cs * 512 + g          # member m at featbase + 64*m
    return lane.astype(np.int32), featbase.astype(np.int64), outcol.astype(np.int64)


def kernel(xyz, xyz_batch_cnt, point_features, conv_w, bn_gamma, bn_beta,
           bn_mean, bn_var, _emulate=False):
    xyz = np.asarray(xyz, dtype=np.float32)
    cnt = np.asarray(xyz_batch_cnt, dtype=np.int64)
    pf = np.asarray(point_features, dtype=np.float32)
    conv_w = np.asarray(conv_w, dtype=np.float32)
    bn_gamma = np.asarray(bn_gamma, dtype=np.float32)
    bn_beta = np.asarray(bn_beta, dtype=np.float32)
    bn_mean = np.asarray(bn_mean, dtype=np.float32)
    bn_var = np.asarray(bn_var, dtype=np.float32)
    N = xyz.shape[0]

    # --- batch ids exactly like jnp.repeat(..., total_repeat_length=N) ---
    ids = np.repeat(np.arange(B), np.maximum(cnt, 0))
    if ids.shape[0] < N:
        pad_val = ids[-1] if ids.shape[0] else 0
        ids = np.concatenate([ids, np.full(N - ids.shape[0], pad_val, np.int64)])
    ids = ids[:N]

    # --- pillar index math, mirroring the reference in f32 ---
    ix = np.clip(np.floor((xyz[:, 0] - X_MIN) / BEV).astype(np.int32), 0, W - 1)
    iy = np.clip(np.floor((xyz[:, 1] - Y_MIN) / BEV).astype(np.int32), 0, H - 1)
    pid_local = iy.astype(np.int64) * W + ix.astype(np.int64)
    cx = (ix.astype(np.float32) + np.float32(0.5)) * BEV + X_MIN
    cy = (iy.astype(np.float32) + np.float32(0.5)) * BEV + Y_MIN
    feats = np.empty((N, 32), dtype=np.float32)
    feats[:, 0] = xyz[:, 0] - cx
    feats[:, 1] = xyz[:, 1] - cy
    feats[:, 2] = xyz[:, 2]
    feats[:, 3:] = pf

    # --- BN fold (scale only; bias applied on host after max) ---
    s = bn_gamma / np.sqrt(bn_var + EPS)
    wt32 = (conv_w * s[:, None]).T                          # [32, 64]
    b2 = bn_beta - bn_mean * s                              # [64]
    wblk = np.zeros((128, 128), dtype=np.float32)
    wblk[0:32, 0:64] = wt32
    wblk[32:64, 64:128] = wt32
    wblk[64:96, 0:64] = wt32
    wblk[96:128, 64:128] = wt32
    wblk_bf = wblk.astype(BF16)

    # --- per-core grouping ---
    bounds = np.searchsorted(ids, np.arange(B + 1))
    cores = []
    max_groups = 0
    for c in range(B):
        lo, hi = int(bounds[c]), int(bounds[c + 1])
        pidc = pid_local[lo:hi]
        counts = np.bincount(pidc, minlength=NPIL).astype(np.int64)
        src, n_ov, ov_pillar, n_groups = _group_layout(pidc, counts)
        cores.append((lo, hi, src, n_ov, ov_pillar, n_groups, counts))
        max_groups = max(max_groups, n_groups)

    NB = max(1, math.ceil(max_groups / 256))
    lane_map, featbase_map, outcol_map = _slot_maps(NB)
    ncols = NB * 512

    # --- pack per-core inputs ---
    in_maps = []
    emu_feats = []
    for c in range(B):
        lo, hi, src, n_ov, ov_pillar, n_groups, counts = cores[c]
        fc = feats[lo:hi]
        if fc.shape[0] == 0:
            fc = np.zeros((1, 32), dtype=np.float32)
        lanes = lane_map[:n_groups]
        fcol = (featbase_map[:n_groups, None] + 64 * np.arange(GK)[None, :])
        fdev = np.zeros((128, ncols), dtype=BF16)
        for L in range(4):
            P = np.zeros(ncols, dtype=np.int64)
            mask = lanes == L
            P[fcol[mask].ravel()] = src[:n_groups][mask].ravel()
            fdev[L * 32:(L + 1) * 32, :] = \
                np.ascontiguousarray(fc[P].T).astype(BF16)
        in_maps.append({"feats": fdev, "wblk": wblk_bf})
        if _emulate:
            emu_feats.append(fdev)

    # --- run (device or numpy emulation) ---
    if _emulate:
        results = []
        for c in range(B):
            fd = emu_feats[c].astype(np.float32)           # [128, ncols]
            o = np.zeros((128, NB * 128), dtype=np.float32)
            for pr in range(2):
                h = wblk[pr * 64:pr * 64 + 64, :].T @ fd[pr * 64:pr * 64 + 64, :]
                # h: [128, ncols]; reduce over members (stride-64 blocks)
                hv = h.reshape(128, NB, 8, 64).max(axis=2)  # [128, NB, 64]
                hv = hv.astype(BF16).astype(np.float32)
                for cs in range(NB):
                    k = cs // 4
                    n = min(4, NB - 4 * k)
                    base = 512 * k + pr * 64 * n + (cs - 4 * k) * 64
                    o[:, base:base + 64] = hv[:, cs, :]
            results.append({"outr": o.astype(BF16)})
    else:
        from concourse.bass_utils import run_bass_kernel_spmd
        key = (NB, PATTERN)
        if key not in _prog_cache:
            _prog_cache[key] = _build_program(NB, PATTERN)
        nc = _prog_cache[key]
        _debug_state["nc"] = nc
        _debug_state["in_maps"] = in_maps
        res = run_bass_kernel_spmd(nc, in_maps, core_ids=list(range(B)))
        results = res.results

    # --- unpack + combine ---
    out_full = np.zeros((B * NPIL, C_OUT), dtype=np.float32)
    for c in range(B):
        lo, hi, src, n_ov, ov_pillar, n_groups, counts = cores[c]
        o = np.asarray(results[c]["outr"]).astype(np.float32)  # [128, NB*128]
        lanes = lane_map[:n_groups]
        ocols = outcol_map[:n_groups]
        rows = np.empty((n_groups, 64), dtype=np.float32)
        for lb in range(2):
            mask = (lanes % 2) == lb
            rows[mask] = o[lb * 64:(lb + 1) * 64, ocols[mask]].T
        main = rows[:NPIL].copy()
        if n_ov:
            ov = rows[NPIL:NPIL + n_ov]
            runs = np.flatnonzero(np.diff(ov_pillar, prepend=-1))
            red = np.maximum.reduceat(ov, runs, axis=0)
            upid = ov_pillar[runs]
            main[upid] = np.maximum(main[upid], red)
        outc = np.maximum(main + b2[None, :], np.float32(0.0))
        outc[counts == 0] = 0.0
        out_full[c * NPIL:(c + 1) * NPIL] = outc
    return out_full
